# revision 9
# baseline (speedup 1.0000x reference)
"""Local contrast normalization (9x9 Gaussian) Trainium2 Bass kernel.

Input x: [64, 512, 512, 1] f32. Output same shape:
    mean = conv2d_same(x, g9x9)
    d    = x - mean
    s    = conv2d_same(d*d, g9x9)
    norm = sqrt(s); keep = norm > 0.5
    out  = where(keep, d / norm, d)

Strategy (pure data parallel, 8 images per core on 8 cores):
  Each 512x512 image is processed in 5 row-windows of 112 output rows.
  Each 2D 9x9 conv is 9 accumulated PE matmuls (one per horizontal tap dj):
  the stationary [K<=128, M<=128] operand is a banded matrix carrying all 9
  vertical taps, and the horizontal shift comes from the rhs free-dim offset
  into a zero-margin-padded SBUF tile. The "x - mean" subtraction is folded
  into the dj=4 matmul by adding a (row-aligned) identity to the banded
  matrix, so PSUM directly holds d. Conv operands are bf16 (1 cycle/row
  streaming + fast FWL weight loads); accumulation and the elementwise
  tail (square, sqrt, reciprocal, threshold blend) stay fp32.
"""

import sys

sys.path.insert(0, "/opt/trn_rl_repo")

import numpy as np

H = W = 512
IMGS_PER_CORE = 8
N_CORES = 8
CHUNK = 112  # output rows per window
THRSHLD = 0.5


def _gauss2d():
    # replicate reference._gauss_kernel exactly
    sigmah = 9 / 6.0
    ii = np.arange(9, dtype=np.float64)
    r2 = (ii[:, None] - 4.5) ** 2 + (ii[None, :] - 4.5) ** 2
    g = np.exp(-r2 / (2.0 * sigmah)).astype(np.float32)
    g = g / g.sum()
    return g  # [9(dv), 9(dj)]


def _windows():
    out = []
    for c in range((H + CHUNK - 1) // CHUNK):
        O0, O1 = CHUNK * c, min(CHUNK * c + CHUNK, H)
        D0, D1 = max(0, O0 - 4), min(H, O1 + 4)
        X0, X1 = max(0, D0 - 4), min(H, D1 + 4)
        out.append((O0, O1, D0, D1, X0, X1))
    return out


WINDOWS = _windows()
N_WIN = len(WINDOWS)
# conv1 lhsT variant: 0 for the first window (X0==D0), 1 otherwise (X0==D0-4)
WTYPE_OF = [0 if c == 0 else 1 for c in range(N_WIN)]


def _gen_weights():
    """W1[vt*9+dj]: [128,128] f32, conv1 lhsT (computes d = x - mean directly,
    identity folded into the dj=4 center tap). Entry [k,m] couples input row
    (X0+k) to output row (D0+m); X0-D0 is 0 (first window) or 4 (others).
    W2[dj]: [128,128] f32, conv2 lhsT: pure relative band g[k-m+4, dj] —
    window-independent; edge rows of each window are partial sums that the
    output DMA never reads."""
    g = _gauss2d()
    W1 = np.zeros((18, 128, 128), np.float32)
    for vt, xd in enumerate([0, 4]):  # xd = D0 - X0
        for dj in range(9):
            A = np.zeros((128, 128), np.float32)
            for k in range(128):
                for m in range(128):
                    dv = k - m - xd + 4
                    v = 0.0
                    if 0 <= dv <= 8:
                        v = -g[dv, dj]
                    if dj == 4 and k - m == xd:
                        v += 1.0
                    A[k, m] = v
            W1[vt * 9 + dj] = A
    W2 = np.zeros((9, 128, 128), np.float32)
    for dj in range(9):
        B = np.zeros((128, 128), np.float32)
        for k in range(128):
            for m in range(128):
                dv = k - m + 4
                if 0 <= dv <= 8:
                    B[k, m] = g[dv, dj]
        W2[dj] = B
    return W1, W2


def _build_program():
    import concourse.bass as bass
    import concourse.bacc as bacc
    import concourse.tile as tile
    from concourse import mybir

    f32 = mybir.dt.float32
    bf16 = mybir.dt.bfloat16

    nc = bacc.Bacc("TRN2", target_bir_lowering=False, debug=False,
                   num_devices=N_CORES)

    rows = IMGS_PER_CORE * H
    x_dram = nc.dram_tensor("x", [rows, W], bf16, kind="ExternalInput")
    w1_dram = nc.dram_tensor("w1", [128, 18 * 128], bf16, kind="ExternalInput")
    w2_dram = nc.dram_tensor("w2", [128, 9 * 128], bf16, kind="ExternalInput")
    o_dram = nc.dram_tensor("out", [rows, W], f32, kind="ExternalOutput")

    with tile.TileContext(nc) as tc:
        with (
            tc.tile_pool(name="wpool", bufs=1) as wpool,
            tc.tile_pool(name="xpool", bufs=4) as xpool,
            tc.tile_pool(name="dpool", bufs=4) as dpool,
            tc.tile_pool(name="spool", bufs=3) as spool,
            tc.tile_pool(name="opool", bufs=4) as opool,
            tc.tile_pool(name="ps1", bufs=4, space=bass.MemorySpace.PSUM) as ps1,
            tc.tile_pool(name="ps2", bufs=4, space=bass.MemorySpace.PSUM) as ps2,
        ):
            w1_sb = wpool.tile([128, 18, 128], bf16)
            w2_sb = wpool.tile([128, 9, 128], bf16)
            nc.sync.dma_start(
                w1_sb[:].rearrange("k v m -> k (v m)"), w1_dram.ap()
            )
            nc.sync.dma_start(
                w2_sb[:].rearrange("k v m -> k (v m)"), w2_dram.ap()
            )
            eps_sb = wpool.tile([128, 1], f32)
            nc.vector.memset(eps_sb[:], 1e-12)

            for i in range(IMGS_PER_CORE):
                for c in range(N_WIN):
                    O0, O1, D0, D1, X0, X1 = WINDOWS[c]
                    nX, nD, nO = X1 - X0, D1 - D0, O1 - O0
                    off2 = O0 - D0
                    vt = WTYPE_OF[c]
                    R = slice(0, nD)

                    x_win = xpool.tile([128, 520], bf16, tag="xwin")
                    nc.gpsimd.memset(x_win[0:nX, 0:4], 0.0)
                    nc.gpsimd.memset(x_win[0:nX, 516:520], 0.0)
                    nc.sync.dma_start(
                        x_win[0:nX, 4:516],
                        x_dram.ap()[i * H + X0 : i * H + X1, :],
                    )

                    psum1 = ps1.tile([128, 512], f32, tag="d")
                    for dj in range(9):
                        nc.tensor.matmul(
                            psum1[0:nD, :],
                            w1_sb[0:nX, vt * 9 + dj, 0:nD],
                            x_win[0:nX, dj : dj + 512],
                            start=(dj == 0),
                            stop=(dj == 8),
                        )

                    dsq = dpool.tile([128, 520], bf16, tag="dsq")
                    nc.gpsimd.memset(dsq[0:nD, 0:4], 0.0)
                    nc.gpsimd.memset(dsq[0:nD, 516:520], 0.0)
                    nc.scalar.activation(
                        dsq[0:nD, 4:516],
                        psum1[0:nD, :],
                        mybir.ActivationFunctionType.Square,
                    )

                    psum2 = ps2.tile([128, 512], f32, tag="s")
                    for dj in range(9):
                        nc.tensor.matmul(
                            psum2[0:nD, :],
                            w2_sb[0:nD, dj, 0:nD],
                            dsq[0:nD, dj : dj + 512],
                            start=(dj == 0),
                            stop=(dj == 8),
                        )

                    norm = spool.tile([128, 512], f32, tag="norm")
                    nc.scalar.activation(
                        norm[R, :],
                        psum2[R, :],
                        mybir.ActivationFunctionType.Sqrt,
                        bias=eps_sb[R, :],
                    )
                    r = spool.tile([128, 512], f32, tag="r")
                    nc.vector.reciprocal_approx_fast(r[R, :], norm[R, :])
                    mask = spool.tile([128, 512], f32, tag="mask")
                    nc.vector.tensor_scalar(
                        mask[R, :], norm[R, :], THRSHLD, None,
                        mybir.AluOpType.is_gt,
                    )
                    t1 = spool.tile([128, 512], f32, tag="t1")
                    nc.vector.scalar_tensor_tensor(
                        t1[R, :], r[R, :], 1.0, mask[R, :],
                        mybir.AluOpType.subtract, mybir.AluOpType.mult,
                    )
                    outt = opool.tile([128, 512], f32, tag="out")
                    nc.vector.scalar_tensor_tensor(
                        outt[R, :], t1[R, :], 1.0, psum1[R, :],
                        mybir.AluOpType.add, mybir.AluOpType.mult,
                    )
                    nc.sync.dma_start(
                        o_dram.ap()[i * H + O0 : i * H + O1, :],
                        outt[off2 : off2 + nO, :],
                    )

    nc.compile()
    return nc


_NC = None


def _get_nc():
    global _NC
    if _NC is None:
        _NC = _build_program()
    return _NC


def _run(x_full, trace=False, **kw):
    from concourse import bass_utils

    nc = _get_nc()
    W1, W2 = _gen_weights()
    import ml_dtypes

    bf = ml_dtypes.bfloat16
    x_full = np.asarray(x_full, dtype=np.float32).reshape(64, H, W)
    W1b = np.ascontiguousarray(W1.transpose(1, 0, 2).reshape(128, 18 * 128)).astype(bf)
    W2b = np.ascontiguousarray(W2.transpose(1, 0, 2).reshape(128, 9 * 128)).astype(bf)
    in_maps = []
    for core in range(N_CORES):
        shard = np.ascontiguousarray(
            x_full[core * IMGS_PER_CORE : (core + 1) * IMGS_PER_CORE].reshape(
                IMGS_PER_CORE * H, W
            )
        ).astype(bf)
        in_maps.append({"x": shard, "w1": W1b, "w2": W2b})
    res = bass_utils.run_bass_kernel_spmd(
        nc, in_maps, core_ids=list(range(N_CORES)), trace=trace, **kw
    )
    out = np.concatenate(
        [r["out"].reshape(IMGS_PER_CORE, H, W) for r in res.results], axis=0
    )
    return out.reshape(64, H, W, 1), res


def kernel(x):
    out, _ = _run(x)
    return out
